# revision 15
# baseline (speedup 1.0000x reference)
"""GCN (2-layer GCNConv + linear head) distributed over 8 TRN2 NeuronCores.

v3 strategy (dst-partitioned graph parallel, fp16 hot path, 4 SWDGE queues):
  - Nodes partitioned into 8 contiguous ranges; core c owns the scatter
    destinations for its range. Per-edge messages gathered from a node-major
    fp16 table in DRAM via dma_gather (256B rows) round-robined over 4 SWDGE
    queues (4x the single-queue ring drain rate).
  - GCN normalization is folded into the data so the scatter weights are
    pure 0/1: the gather table holds x' = x * dinv (host) resp.
    h1' = dinv * relu(...) (device), and the remaining per-dst dinv factor
    rides the transform activation's per-partition scale. Self-loops become
    plain edges (constant diagonal one-hot, no DVE work).
  - Scatter via one-hot matmul on the TensorEngine: for each chunk of 128
    gathered edges, rhs[e, d] = (d == dst_e) is ONE fused DVE tensor_scalar
    (is_equal against a per-partition scalar); psum[f, d] += chunk.T @ rhs.
  - Cells are (supertile=256 dsts, window=src range); chunks padded to 128
    per cell with counts maxed across cores (shared SPMD schedule). The
    idx/dst streams are shared by both layers and kept resident in SBUF.
  - Per-block pipeline: up to 5 supertile PSUM accumulators (1 bank each)
    live across the block's windows; transforms chase the accumulation.
    Both layers' transforms are NODE-major (lhsT=agg, rhs=W) so the
    per-dst dinv scale is a per-partition activation scale; layer-2's
    linear head is one DVE tensor_tensor_reduce per tile.
  - One AllGather (fp16, ~25MB) between the layers.

Host prep (degree/norm computation, edge bucketing, padding) is numpy inside
kernel(); the device kernel is a single static SPMD program.
"""

import math
import os
import sys

import numpy as np

for _p in ("/opt/trn_rl_repo",):
    if _p not in sys.path and os.path.isdir(_p):
        sys.path.insert(0, _p)

F = 128  # feature/hidden width
STW = 256  # dsts per supertile (2 tiles)


class Cfg:
    def __init__(self, n_cores=8, nodes_real_per_core=12500, n_edges=1_600_000,
                 n_windows=4, bst_max=5, gather_rows=8192):
        self.C = n_cores
        self.NR = nodes_real_per_core
        self.T = (self.NR + 127) // 128          # dst tiles per core
        assert self.T % 2 == 0, "supertile=2 tiles needs even T"
        self.NST = self.T // 2                   # supertiles per core
        self.S = self.T * 128                    # node slots per core
        self.NSLOT = self.C * self.S             # global slot count
        self.NW = n_windows
        assert self.NSLOT % self.NW == 0
        self.WIN = self.NSLOT // self.NW         # rows per gather window
        assert self.WIN <= 32767, "dma_gather idx is int16"
        self.BSTM = bst_max                      # max supertiles per block
        self.BLOCKS = []
        r = self.NST
        while r > 0:
            self.BLOCKS.append(min(bst_max, r))
            r -= min(bst_max, r)
        self.GB = gather_rows                    # max rows per dma_gather
        assert self.GB % 128 == 0
        self.N = self.C * self.NR                # real node count
        self.E = n_edges


FULL = Cfg()


# ------------------------------------------------------------- host prep ----

def prepare(cfg: Cfg, x, edge_index):
    """Compute per-core device inputs + the shared static schedule."""
    C, NR, T, S, NW, WIN = cfg.C, cfg.NR, cfg.T, cfg.S, cfg.NW, cfg.WIN
    NST, GB = cfg.NST, cfg.GB
    N = cfg.N
    src = np.asarray(edge_index[0], dtype=np.int64)
    dst = np.asarray(edge_index[1], dtype=np.int64)
    x = np.asarray(x, dtype=np.float32)

    deg = np.bincount(dst, minlength=N).astype(np.float64) + 1.0  # + self loop
    dinv = (1.0 / np.sqrt(deg)).astype(np.float32)

    core_of = dst // NR
    s_slot = S * (src // NR) + (src % NR)
    d_in_core = dst % NR
    st_of = d_in_core // STW
    d_loc = (d_in_core - st_of * STW).astype(np.float32)  # 0..255
    w_of = s_slot // WIN

    order = np.lexsort((s_slot, w_of, st_of, core_of))
    s_slot, d_loc = s_slot[order], d_loc[order]
    core_s, st_s, w_s = core_of[order], st_of[order], w_of[order]

    cell = ((core_s * NST + st_s) * NW + w_s).astype(np.int64)
    counts = np.bincount(cell, minlength=C * NST * NW).reshape(C, NST, NW)
    Kcell = np.ceil(counts / 128.0).astype(np.int64).max(axis=0)  # [NST, NW]
    cell_starts = np.zeros(C * NST * NW + 1, dtype=np.int64)
    np.cumsum(np.bincount(cell, minlength=C * NST * NW), out=cell_starts[1:])

    # ---- shared schedule: blocks x windows -> calls of <= GB/128 chunks ----
    chunk_pos = np.zeros((NST, NW), dtype=np.int64)
    calls = []  # (block_idx, w, col0_chunk, [(st, k)...]) per gather call
    acc = 0
    st0 = 0
    for bi, bsz in enumerate(cfg.BLOCKS):
        sts = range(st0, st0 + bsz)
        for w in range(NW):
            pend = []
            for st in sts:
                chunk_pos[st, w] = acc + len(pend)
                pend += [(st, k) for k in range(int(Kcell[st, w]))]
            i = 0
            while i < len(pend):
                n = min(GB // 128, len(pend) - i)
                calls.append((bi, w, acc + i, pend[i:i + n]))
                i += n
            acc += len(pend)
        st0 += bsz
    Ctot = acc
    assert Ctot == int(Kcell.sum())

    # last (w, k) per supertile for the psum stop flag
    last_of_st = {}
    for st in range(NST):
        ws = [w for w in range(NW) if Kcell[st, w] > 0]
        if ws:
            w = ws[-1]
            last_of_st[st] = (w, int(Kcell[st, w]) - 1)

    # ---- per-core streams ----
    per_core = []
    for c in range(C):
        idxs = np.zeros(Ctot * 128, dtype=np.int16)
        dstv = np.full(Ctot * 128, -1.0, dtype=np.float32)
        for st in range(NST):
            for w in range(NW):
                K = int(Kcell[st, w])
                if K == 0:
                    continue
                ci = (c * NST + st) * NW + w
                e0, e1 = cell_starts[ci], cell_starts[ci + 1]
                n = int(e1 - e0)
                off = int(chunk_pos[st, w]) * 128
                idxs[off:off + n] = (s_slot[e0:e1] - w_s[e0:e1] * WIN).astype(np.int16)
                dstv[off:off + n] = d_loc[e0:e1]

        # idx wrapped [128, Ctot*8]: idx i -> [i%16, i//16], replicated x8
        idx_w = np.tile(idxs.reshape(-1, 16).T, (8, 1)).copy()
        dst_t = dstv.reshape(Ctot, 128).T.copy()

        dv = np.zeros(S, dtype=np.float32)
        dv[:NR] = dinv[c * NR:(c + 1) * NR]
        dinvc_t = dv.reshape(T, 128).T.copy()       # [128, T] dinv
        dinv2_t = (dv * dv).reshape(T, 128).T.copy()  # [128, T] dinv^2

        per_core.append(dict(idx_w=idx_w, dst_t=dst_t,
                             dinvc_t=dinvc_t, dinv2_t=dinv2_t))

    # node tables in slot space, pre-scaled by dinv (fp16)
    x_slot = np.zeros((cfg.NSLOT, F), dtype=np.float16)
    sl = S * (np.arange(N) // NR) + (np.arange(N) % NR)
    x_slot[sl] = (x * dinv[:, None]).astype(np.float16)
    for c in range(C):
        per_core[c]["xtab"] = x_slot
        per_core[c]["xloc"] = x_slot[c * S:(c + 1) * S].copy()

    layout = dict(Kcell=Kcell, calls=calls, chunk_pos=chunk_pos,
                  last_of_st=last_of_st, Ctot=Ctot)
    return layout, per_core


# ---------------------------------------------------------------- builder ----

def build_nc(cfg: Cfg, layout, zero_b1=True, zero_b2=True):
    import concourse.bacc as bacc
    import concourse.mybir as mybir
    import concourse.tile as tile

    dtf = mybir.dt.float32
    dth = mybir.dt.float16
    Relu = mybir.ActivationFunctionType.Relu
    Copy = mybir.ActivationFunctionType.Copy
    EQ = mybir.AluOpType.is_equal
    MUL = mybir.AluOpType.mult
    ADD = mybir.AluOpType.add
    MAX = mybir.AluOpType.max

    C, T, S, NW, WIN, GB = cfg.C, cfg.T, cfg.S, cfg.NW, cfg.WIN, cfg.GB
    NST = cfg.NST
    Kcell, calls, chunk_pos, last_of_st, Ctot = (
        layout["Kcell"], layout["calls"], layout["chunk_pos"],
        layout["last_of_st"], layout["Ctot"])

    nc = bacc.Bacc("TRN2", target_bir_lowering=False, debug=False,
                   num_devices=C, num_swdge_queues=4)

    xtab_d = nc.dram_tensor("xtab", [cfg.NSLOT, F], dth, kind="ExternalInput").ap()
    xloc_d = nc.dram_tensor("xloc", [S, F], dth, kind="ExternalInput").ap()
    idx_d = nc.dram_tensor("idx_w", [128, Ctot * 8], mybir.dt.int16,
                           kind="ExternalInput").ap()
    dst_d = nc.dram_tensor("dst_t", [128, Ctot], dtf, kind="ExternalInput").ap()
    dinvc_d = nc.dram_tensor("dinvc_t", [128, T], dtf, kind="ExternalInput").ap()
    dinv2_d = nc.dram_tensor("dinv2_t", [128, T], dtf, kind="ExternalInput").ap()
    iota_d = nc.dram_tensor("iota256", [128, STW], dth, kind="ExternalInput").ap()
    diag_d = nc.dram_tensor("diag2", [128, 2 * STW], dth, kind="ExternalInput").ap()
    W1_d = nc.dram_tensor("W1", [F, F], dth, kind="ExternalInput").ap()
    W2_d = nc.dram_tensor("W2", [F, F], dth, kind="ExternalInput").ap()
    Wlbc_d = nc.dram_tensor("Wlbc", [128, F], dth, kind="ExternalInput").ap()
    b1bc_d = nc.dram_tensor("b1bc", [128, F], dth, kind="ExternalInput").ap()
    b2bc_d = nc.dram_tensor("b2bc", [128, F], dth, kind="ExternalInput").ap()
    blc_d = nc.dram_tensor("blcol", [128, 1], dtf, kind="ExternalInput").ap()
    out_d = nc.dram_tensor("out", [S, 1], dtf, kind="ExternalOutput").ap()

    with tile.TileContext(nc) as tc:
        with (
            tc.tile_pool(name="const", bufs=1) as const,
            tc.tile_pool(name="sb", bufs=2) as sb,
            tc.tile_pool(name="ohp", bufs=6) as ohp,
            tc.tile_pool(name="aggp", bufs=1, space="PSUM") as aggp,
            tc.tile_pool(name="pt", bufs=1, space="PSUM") as pt,
            tc.tile_pool(name="dram", bufs=1, space="DRAM") as dram,
        ):
            iota256 = const.tile([128, STW], dth)
            nc.sync.dma_start(iota256[:], iota_d)
            diag2 = const.tile([128, 2 * STW], dth)
            nc.sync.dma_start(diag2[:], diag_d)
            W1s = const.tile([F, F], dth)
            nc.sync.dma_start(W1s[:], W1_d)
            W2s = const.tile([F, F], dth)
            nc.sync.dma_start(W2s[:], W2_d)
            Wlbc = const.tile([128, F], dth)
            nc.sync.dma_start(Wlbc[:], Wlbc_d)
            b1bc = const.tile([128, F], dth)
            nc.sync.dma_start(b1bc[:], b1bc_d)
            b2bc = const.tile([128, F], dth)
            nc.sync.dma_start(b2bc[:], b2bc_d)
            blcol = const.tile([128, 1], dtf)
            nc.sync.dma_start(blcol[:], blc_d)
            dinvcs = const.tile([128, T], dtf)
            nc.sync.dma_start(dinvcs[:], dinvc_d)
            dinv2s = const.tile([128, T], dtf)
            nc.sync.dma_start(dinv2s[:], dinv2_d)
            dsts = const.tile([128, Ctot], dtf)
            nc.sync.dma_start(dsts[:], dst_d)
            idxs = const.tile([128, Ctot * 8], mybir.dt.int16)
            nc.sync.dma_start(idxs[:], idx_d)

            outcols = const.tile([128, T], dtf)

            h1_loc = dram.tile([S, F], dth)
            ag_tab = dram.tile([cfg.NSLOT, F], dth, addr_space="Shared")

            qctr = 0
            for layer in range(2):
                table = xtab_d if layer == 0 else ag_tab[:]
                loc = xloc_d if layer == 0 else h1_loc[:]
                Ws = W1s if layer == 0 else W2s
                zb = zero_b1 if layer == 0 else zero_b2
                bbc = b1bc if layer == 0 else b2bc

                st0 = 0
                for bi, bsz in enumerate(cfg.BLOCKS):
                    blk = sb.tile([128, 2 * cfg.BSTM, F], dth, tag="selfblk")
                    nc.sync.dma_start(
                        blk[:, :2 * bsz, :],
                        loc[st0 * STW:st0 * STW + bsz * STW, :]
                        .rearrange("(bt p) f -> p bt f", p=128))

                    psums = {}
                    for sti in range(bsz):
                        st = st0 + sti
                        ps = aggp.tile([128, STW], dtf, tag=f"agg{sti}",
                                       name=f"agg{sti}")
                        psums[st] = ps
                        for i in range(2):
                            is_last = (st not in last_of_st) and i == 1
                            nc.tensor.matmul(
                                out=ps[:], lhsT=blk[:, 2 * sti + i, :],
                                rhs=diag2[:, i * STW:(i + 1) * STW],
                                start=(i == 0), stop=is_last,
                                skip_group_check=True)

                    for w in range(NW):
                        for call in [cl for cl in calls
                                     if cl[0] == bi and cl[1] == w]:
                            _, _, col0, chunks = call
                            ncols = len(chunks)
                            q = qctr % 4
                            qctr += 1
                            xb = sb.tile([128, GB // 128, F], dth,
                                         tag=f"xb{q}", bufs=2)
                            nc.gpsimd.dma_gather(
                                xb[:, :ncols, :],
                                table[w * WIN:(w + 1) * WIN, :],
                                idxs[:, col0 * 8:(col0 + ncols) * 8],
                                ncols * 128, ncols * 128, F,
                                single_packet=False, queue_num=q)
                            for j, (st, k) in enumerate(chunks):
                                gch = int(chunk_pos[st, w]) + k
                                oh = ohp.tile([128, STW], dth, tag="oh")
                                nc.vector.tensor_scalar(
                                    out=oh[:], in0=iota256[:],
                                    scalar1=dsts[:, gch:gch + 1],
                                    scalar2=1.0, op0=EQ, op1=MUL)
                                is_last = last_of_st.get(st) == (w, k)
                                nc.tensor.matmul(
                                    out=psums[st][:], lhsT=xb[:, j, :],
                                    rhs=oh[:], start=False, stop=is_last,
                                    skip_group_check=True)

                    # block transforms (node-major both layers)
                    for sti in range(bsz):
                        st = st0 + sti
                        aggb = sb.tile([128, STW], dth, tag="aggb")
                        nc.vector.tensor_copy(out=aggb[:], in_=psums[st][:])
                        pp = pt.tile([128, STW], dtf, tag="pp", bufs=2)
                        for i in range(2):
                            sl = slice(i * 128, (i + 1) * 128)
                            nc.tensor.matmul(
                                out=pp[:, sl], lhsT=aggb[:, sl], rhs=Ws[:],
                                start=True, stop=True, skip_group_check=True)
                        for i in range(2):
                            t = 2 * st + i
                            sl = slice(i * 128, (i + 1) * 128)
                            if layer == 0:
                                h1n = sb.tile([128, F], dth, tag="h1n")
                                if zb:
                                    # h1' = dinv*relu(dinv*z) = relu(dinv^2 z)
                                    nc.scalar.activation(
                                        out=h1n[:], in_=pp[:, sl], func=Relu,
                                        scale=dinv2s[:, t:t + 1])
                                else:
                                    u = sb.tile([128, F], dth, tag="u")
                                    nc.scalar.activation(
                                        out=u[:], in_=pp[:, sl], func=Copy,
                                        scale=dinvcs[:, t:t + 1])
                                    nc.vector.tensor_tensor(
                                        out=u[:], in0=u[:], in1=b1bc[:],
                                        op=ADD)
                                    nc.scalar.activation(
                                        out=h1n[:], in_=u[:], func=Relu,
                                        scale=dinvcs[:, t:t + 1])
                                nc.sync.dma_start(
                                    h1_loc[t * 128:(t + 1) * 128, :], h1n[:])
                            else:
                                h2n = sb.tile([128, F], dth, tag="h2n")
                                if zb:
                                    nc.scalar.activation(
                                        out=h2n[:], in_=pp[:, sl], func=Relu,
                                        scale=dinvcs[:, t:t + 1])
                                else:
                                    u = sb.tile([128, F], dth, tag="u")
                                    nc.scalar.activation(
                                        out=u[:], in_=pp[:, sl], func=Copy,
                                        scale=dinvcs[:, t:t + 1])
                                    nc.vector.tensor_tensor(
                                        out=u[:], in0=u[:], in1=b2bc[:],
                                        op=ADD)
                                    nc.vector.tensor_scalar(
                                        out=h2n[:], in0=u[:], scalar1=0.0,
                                        scalar2=None, op0=MAX)
                                scr = sb.tile([128, F], dth, tag="scr")
                                nc.vector.tensor_tensor(
                                    out=scr[:], in0=h2n[:], in1=Wlbc[:],
                                    op=MUL)
                                nc.vector.tensor_reduce(
                                    out=outcols[:, t:t + 1], in_=scr[:],
                                    axis=mybir.AxisListType.X, op=ADD)
                    st0 += bsz

                if layer == 0:
                    nc.gpsimd.collective_compute(
                        "AllGather", mybir.AluOpType.bypass,
                        replica_groups=[list(range(C))],
                        ins=[h1_loc[:]], outs=[ag_tab[:]])

            nc.vector.tensor_scalar(out=outcols[:], in0=outcols[:],
                                    scalar1=blcol[:], scalar2=None, op0=ADD)
            nc.sync.dma_start(
                out_d.rearrange("(t p) o -> p (t o)", p=128), outcols[:])

    nc.compile()
    return nc


# ------------------------------------------------------------------ entry ----

def make_in_maps(cfg, per_core, W1, b1, W2, b2, Wl, bl):
    iota256 = np.tile(np.arange(STW, dtype=np.float16), (128, 1))
    diag2 = np.zeros((128, 2 * STW), dtype=np.float16)
    for i in range(2):
        for p in range(128):
            diag2[p, i * STW + p + 128 * i] = 1.0
    maps = []
    for c in range(cfg.C):
        pc = per_core[c]
        maps.append(dict(
            xtab=pc["xtab"], xloc=pc["xloc"], idx_w=pc["idx_w"],
            dst_t=pc["dst_t"], dinvc_t=pc["dinvc_t"], dinv2_t=pc["dinv2_t"],
            iota256=iota256, diag2=diag2,
            W1=np.asarray(W1, np.float16), W2=np.asarray(W2, np.float16),
            Wlbc=np.tile(np.asarray(Wl, np.float16).reshape(1, F), (128, 1)),
            b1bc=np.tile(np.asarray(b1, np.float16).reshape(1, F), (128, 1)),
            b2bc=np.tile(np.asarray(b2, np.float16).reshape(1, F), (128, 1)),
            blcol=np.full((128, 1), np.asarray(bl, np.float32).reshape(-1)[0],
                          dtype=np.float32),
        ))
    return maps


def run(cfg, x, edge_index, W1, b1, W2, b2, Wl, bl, trace=False, nc=None):
    from concourse import bass_utils

    layout, per_core = prepare(cfg, x, edge_index)
    if nc is None:
        nc = build_nc(cfg, layout,
                      zero_b1=not np.any(np.asarray(b1)),
                      zero_b2=not np.any(np.asarray(b2)))
    in_maps = make_in_maps(cfg, per_core, W1, b1, W2, b2, Wl, bl)
    res = bass_utils.run_bass_kernel_spmd(nc, in_maps,
                                          core_ids=list(range(cfg.C)),
                                          trace=trace)
    out = np.concatenate([res.results[c]["out"][:cfg.NR, 0]
                          for c in range(cfg.C)])
    return out.astype(np.float32), res


def kernel(x, edge_index, W1, b1, W2, b2, Wl, bl):
    out, _ = run(FULL, x, edge_index, W1, b1, W2, b2, Wl, bl)
    return out
